# revision 8
# baseline (speedup 1.0000x reference)
"""Causal self-attention on 8 TRN2 NeuronCores.

Problem (hardcoded): B=4, T=2048, C=1024, H=16 heads, D=64.
  qkv = x @ W_in + b_in ; causal softmax attention ; out = y @ W_out + b_out

Sharding: core c handles batch b = c//2 and head-group g = c%2 (8 heads).
Each core computes its partial out-projection (sum over its heads' columns);
the host adds the two partials per batch plus b_out. No device collectives.

Device layout notes:
  - All matmul operands are float32r (full-rate PE, ~1e-4 rel err).
  - x is pre-transposed on host so the projection contraction (over C) has
    C on partitions with natural contiguous DMA loads.
  - q is pre-scaled by 1/sqrt(D) on host (folded into W_q, b_q).
  - Scores are computed transposed: S^T[k, q] = k . q, so softmax exp output
    P^T feeds the PV matmul directly (no on-chip transposes at all).
  - exp without max-subtraction: scores are ~N(0,1); fp32 exp is safe.
  - v gets a ones-column appended per head: the PV matmul then yields the
    softmax denominator in its last output row for free.
"""

import sys

for _p in ("/opt/trn_rl_repo", "/root/.axon_site/_ro/trn_rl_repo"):
    if _p not in sys.path:
        sys.path.append(_p)

import numpy as np

B, T, C = 4, 2048, 1024
H = 16  # total heads
HL = 8  # heads per core
D = 64  # head dim
P = 128
KO = C // P  # 8 contraction chunks
TQ = 512  # query-tile width
NTQ = T // TQ  # 4
NTK = T // P  # 16 key chunks
G2 = 2  # key chunks per exp group
FR = None  # mybir.dt.float32r, set on first build

_CACHE = {}


def _build():
    import concourse.mybir as mybir
    import concourse.tile as tile
    from concourse import bacc

    fr = mybir.dt.float32r
    f32 = mybir.dt.float32

    nc = bacc.Bacc("TRN2", target_bir_lowering=False, debug=False, num_devices=8)

    xT = nc.dram_tensor("xT", [C, T], fr, kind="ExternalInput")
    w_qk = nc.dram_tensor("w_qk", [C, 2 * HL * D], fr, kind="ExternalInput")
    b_qk = nc.dram_tensor("b_qk", [2 * HL * D], f32, kind="ExternalInput")
    w_v = nc.dram_tensor("w_v", [C, HL * D], fr, kind="ExternalInput")
    b_v = nc.dram_tensor("b_v", [HL * D], fr, kind="ExternalInput")
    w_out = nc.dram_tensor("w_out", [HL * D, C], fr, kind="ExternalInput")
    masks = nc.dram_tensor("masks", [P, 896], fr, kind="ExternalInput")
    ones64 = nc.dram_tensor("ones64", [1, D], fr, kind="ExternalInput")
    vones = nc.dram_tensor("vones", [P, NTK * HL], fr, kind="ExternalInput")
    out = nc.dram_tensor("out", [T, C], f32, kind="ExternalOutput")

    FQK = 2 * HL * D  # 1024 (q block then k block)
    FV = HL * D  # 512

    with tile.TileContext(nc) as tc:
        import contextlib

        ctx = contextlib.ExitStack()
        with ctx:
            persist = ctx.enter_context(tc.tile_pool(name="persist", bufs=1))
            qkv_pool = ctx.enter_context(tc.tile_pool(name="qkv", bufs=1))

            # ---- persistent small tensors ----
            w_out_sb = persist.tile([P, 4, C], fr)  # [p, do, n]
            nc.sync.dma_start(w_out_sb, w_out.rearrange("(do p) n -> p do n", p=P))
            b_qk_sb = persist.tile([P, KO], f32)  # per-partition bias per F chunk
            nc.sync.dma_start(b_qk_sb, b_qk.rearrange("(fo p) -> p fo", p=P))
            mask_sb = persist.tile([P, 896], fr)
            nc.sync.dma_start(mask_sb, masks[:])
            ones64_sb = persist.tile([1, D], fr)
            nc.sync.dma_start(ones64_sb, ones64[:])
            bv_bc = persist.tile([P, FV], fr)  # b_v broadcast to all partitions
            nc.sync.dma_start(bv_bc, b_v[None, :].to_broadcast((P, FV)))

            # ---- big persistent activations ----
            qkT = qkv_pool.tile([P, KO, T], fr)  # [p, fo, t] : F = fo*128+p
            v65 = qkv_pool.tile([P, NTK, HL, D + 1], fr)  # v + ones column

            # ones column of v65
            nc.sync.dma_start(
                v65[:, :, :, D],
                vones.rearrange("p (n h) -> p n h", n=NTK),
            )

            # ================= Phase 1: projections =================
            with tc.tile_pool(name="ph1", bufs=1) as ph1, \
                 tc.tile_pool(name="ph1x", bufs=2) as ph1x, \
                 tc.tile_pool(name="ps1", bufs=4, space="PSUM") as ps1:
                wqk_t = ph1.tile([P, KO, FQK], fr)
                nc.sync.dma_start(wqk_t, w_qk.rearrange("(ko p) f -> p ko f", p=P))
                wv_t = ph1.tile([P, KO, FV], fr)
                nc.sync.dma_start(wv_t, w_v.rearrange("(ko p) f -> p ko f", p=P))

                for tq in range(NTQ):
                    xT_k = []
                    for ko in range(KO):
                        t_ = ph1x.tile([P, TQ], fr, tag=f"xT{ko}")
                        nc.sync.dma_start(
                            t_, xT[ko * P : (ko + 1) * P, tq * TQ : (tq + 1) * TQ]
                        )
                        xT_k.append(t_)

                    # qk^T = W_qk^T @ x^T  -> [F, T] tiles [128, 512]
                    for fo in range(KO):
                        ps = ps1.tile([P, TQ], f32, tag="ps_qk")
                        for ko in range(KO):
                            nc.tensor.matmul(
                                ps,
                                wqk_t[:, ko, fo * P : (fo + 1) * P],
                                xT_k[ko],
                                start=(ko == 0),
                                stop=(ko == KO - 1),
                            )
                        # add per-partition bias, write to qkT
                        nc.vector.tensor_scalar(
                            qkT[:, fo, tq * TQ : (tq + 1) * TQ],
                            ps,
                            b_qk_sb[:, fo : fo + 1],
                            None,
                            mybir.AluOpType.add,
                        )

                    # v = x @ W_v (+ b_v) -> natural [T, FV], strided into v65
                    for t4 in range(4):
                        to = 4 * tq + t4
                        ps = ps1.tile([P, FV], f32, tag="ps_v")
                        for ko in range(KO):
                            nc.tensor.matmul(
                                ps,
                                xT_k[ko][:, t4 * P : (t4 + 1) * P],
                                wv_t[:, ko],
                                start=(ko == 0),
                                stop=(ko == KO - 1),
                            )
                        nc.vector.tensor_tensor(
                            v65[:, to, :, :D],
                            ps.rearrange("p (h d) -> p h d", h=HL),
                            bv_bc.rearrange("p (h d) -> p h d", h=HL),
                            mybir.AluOpType.add,
                        )

            # ================= Phase 2: attention =================
            with tc.tile_pool(name="ph2", bufs=1) as ph2, \
                 tc.tile_pool(name="pT_pool", bufs=3) as pT_pool, \
                 tc.tile_pool(name="sm", bufs=2) as sm, \
                 tc.tile_pool(name="ps2", bufs=2, space="PSUM") as ps2, \
                 tc.tile_pool(name="out_pool", bufs=3) as out_pool:
                yT = ph2.tile([P, 4, T], fr)  # [p, do, t] : D_local = do*128+p

                for tq in range(NTQ):
                    nchunks = 4 * (tq + 1)
                    for h in range(HL):
                        pbase = D * (h % 2)
                        qfo = h // 2
                        kfo = 4 + h // 2
                        qT_h = qkT[pbase : pbase + D, qfo, tq * TQ : (tq + 1) * TQ]

                        ps_y = ps2.tile([D + 1, TQ], f32, tag="ps_y")
                        pTs = []
                        # S^T tiles + exp, in groups of G2 chunks
                        for gi in range(nchunks // G2):
                            ps_s = ps2.tile([P, G2 * TQ], f32, tag="ps_s")
                            for c2 in range(G2):
                                i = gi * G2 + c2
                                kT_i = qkT[
                                    pbase : pbase + D, kfo, i * P : (i + 1) * P
                                ]
                                nc.tensor.matmul(
                                    ps_s[:, c2 * TQ : (c2 + 1) * TQ],
                                    kT_i,
                                    qT_h,
                                    start=True,
                                    stop=True,
                                )
                            pT = pT_pool.tile([P, G2 * TQ], fr, tag="pT")
                            nc.scalar.activation(
                                pT, ps_s, mybir.ActivationFunctionType.Exp
                            )
                            # causal mask on diagonal chunks
                            for c2 in range(G2):
                                i = gi * G2 + c2
                                i4 = i - 4 * tq
                                if 0 <= i4 < 4:
                                    off = 384 - 128 * i4
                                    nc.vector.tensor_tensor(
                                        pT[:, c2 * TQ : (c2 + 1) * TQ],
                                        pT[:, c2 * TQ : (c2 + 1) * TQ],
                                        mask_sb[:, off : off + TQ],
                                        mybir.AluOpType.mult,
                                    )
                            pTs.append(pT)
                        # PV accumulation (+ denominator row)
                        for gi in range(nchunks // G2):
                            for c2 in range(G2):
                                i = gi * G2 + c2
                                nc.tensor.matmul(
                                    ps_y,
                                    v65[:, i, h],
                                    pTs[gi][:, c2 * TQ : (c2 + 1) * TQ],
                                    start=(i == 0),
                                    stop=(i == nchunks - 1),
                                )
                        # normalize: rec = 1/denom ; broadcast ; multiply
                        rec = sm.tile([1, TQ], fr, tag="rec")
                        with nc.allow_low_precision(reason="fp32r ~19-bit is plenty"):
                            nc.vector.reciprocal(rec, ps_y[D : D + 1, :])
                        rec_bc = sm.tile([D, TQ], fr, tag="rec_bc")
                        nc.gpsimd.partition_broadcast(rec_bc, rec, channels=D)
                        nc.vector.tensor_tensor(
                            yT[pbase : pbase + D, h // 2, tq * TQ : (tq + 1) * TQ],
                            ps_y[:D, :],
                            rec_bc,
                            mybir.AluOpType.mult,
                        )

                    # ---- out-projection for this tq window ----
                    for ts_ in range(4):
                        t0 = tq * TQ + ts_ * P
                        for n in range(2):
                            ps_o = ps2.tile([P, 512], f32, tag="ps_s")
                            for do in range(4):
                                nc.tensor.matmul(
                                    ps_o,
                                    yT[:, do, t0 : t0 + P],
                                    w_out_sb[:, do, n * 512 : (n + 1) * 512],
                                    start=(do == 0),
                                    stop=(do == 3),
                                )
                            o_sb = out_pool.tile([P, 512], f32, tag="o")
                            nc.vector.tensor_copy(o_sb, ps_o)
                            nc.sync.dma_start(
                                out[t0 : t0 + P, n * 512 : (n + 1) * 512], o_sb
                            )

    nc.compile()
    return nc


def _get_nc():
    if "nc" not in _CACHE:
        _CACHE["nc"] = _build()
    return _CACHE["nc"]


def kernel(x, W_in, b_in, W_out, b_out):
    from concourse.bass_utils import run_bass_kernel_spmd

    x = np.asarray(x, dtype=np.float32)
    W_in = np.asarray(W_in, dtype=np.float32)
    b_in = np.asarray(b_in, dtype=np.float32)
    W_out = np.asarray(W_out, dtype=np.float32)
    b_out = np.asarray(b_out, dtype=np.float32)

    scale = 1.0 / np.sqrt(D)

    # causal mask master: M[p, u] = 1 if u >= p + 384
    u = np.arange(896)[None, :]
    p = np.arange(P)[:, None]
    mask = (u >= p + 384).astype(np.float32)
    ones64_np = np.ones((1, D), np.float32)
    vones_np = np.ones((P, NTK * HL), np.float32)

    in_maps = []
    for c in range(8):
        b, g = c // 2, c % 2
        qc = slice(g * HL * D, (g + 1) * HL * D)
        kc = slice(C + g * HL * D, C + (g + 1) * HL * D)
        vc = slice(2 * C + g * HL * D, 2 * C + (g + 1) * HL * D)
        w_qk = np.concatenate([W_in[:, qc] * scale, W_in[:, kc]], axis=1)
        b_qk = np.concatenate([b_in[qc] * scale, b_in[kc]])
        in_maps.append(
            {
                "xT": np.ascontiguousarray(x[b].T),
                "w_qk": np.ascontiguousarray(w_qk),
                "b_qk": np.ascontiguousarray(b_qk),
                "w_v": np.ascontiguousarray(W_in[:, vc]),
                "b_v": np.ascontiguousarray(b_in[vc]),
                "w_out": np.ascontiguousarray(W_out[g * HL * D : (g + 1) * HL * D, :]),
                "masks": mask,
                "ones64": ones64_np,
                "vones": vones_np,
            }
        )

    global _last_in_maps
    _last_in_maps = in_maps
    nc = _get_nc()
    res = run_bass_kernel_spmd(nc, in_maps, list(range(8)))

    out = np.empty((B, T, C), np.float32)
    for b in range(B):
        out[b] = res.results[2 * b]["out"] + res.results[2 * b + 1]["out"] + b_out
    return out


if __name__ == "__main__":
    rng = np.random.default_rng(0)
    x = rng.standard_normal((B, T, C), dtype=np.float32)
    W_in = rng.standard_normal((C, 3 * C), dtype=np.float32) / np.sqrt(C)
    b_in = np.zeros(3 * C, np.float32)
    W_out = rng.standard_normal((C, C), dtype=np.float32) / np.sqrt(C)
    b_out = np.zeros(C, np.float32)
    y = kernel(x=x, W_in=W_in, b_in=b_in, W_out=W_out, b_out=b_out)
    print("ok", y.shape, y.dtype)
